# revision 3
# baseline (speedup 1.0000x reference)
"""Trainium2 Bass kernel for the GwPFM pairwise field-interaction module.

out[b,d] = sum_{i<j} corr[g_i,g_j] * x[b,i,g_j,d] * x[b,j,g_i,d],
B=2048, F=32, G=8 (g_i = i%8), D=64.

Device algebra (validated vs reference in numpy):
  field i = 8k+g;  A_k[g,h,d] = x[8k+g,h,d];  C_k = sum_{k'>k} A_k';
  T = sum_k A_k
  PF = T * T^swap ;  PL = sum_{k=0..2} C_k * A_k^swap   (^swap = (g,h)->(h,g))
  out = sum_{g,h} alpha*PF + beta*PL,
  alpha = upper(w), beta = upper(w^T - w) + diag(w).
All ops are lane-local on VectorE with strided APs; batch is on partitions.
Sharding: pure data-parallel, 256 batch rows per NeuronCore (x8).
"""

import sys

import numpy as np

B, F, G, D = 2048, 32, 8, 64
NCORES = 8
BC = B // NCORES          # 256
ROWS = F * G * D          # 16384
_CACHE = {}


def _import_concourse():
    try:
        import concourse  # noqa: F401
    except ImportError:
        sys.path.insert(0, "/opt/trn_rl_repo")


def _build():
    _import_concourse()
    from concourse import mybir
    from concourse.bass import Bass

    f32 = mybir.dt.float32
    AL = mybir.AluOpType
    AX = mybir.AxisListType

    nc = Bass("TRN2", target_bir_lowering=False, debug=False)
    x = nc.dram_tensor("x", [BC, ROWS], f32, kind="ExternalInput")
    ab = nc.dram_tensor("ab", [128, 128], f32, kind="ExternalInput")
    out = nc.dram_tensor("out", [BC, D], f32, kind="ExternalOutput")

    xt = [nc.alloc_sbuf_tensor(f"xt{t}", [128, ROWS], f32).ap() for t in range(2)]
    abt = nc.alloc_sbuf_tensor("abt", [128, 128], f32).ap()
    C1 = nc.alloc_sbuf_tensor("C1", [128, 2048], f32).ap()
    C0 = nc.alloc_sbuf_tensor("C0", [128, 2048], f32).ap()
    Tb = nc.alloc_sbuf_tensor("Tb", [128, 2048], f32).ap()
    S1 = nc.alloc_sbuf_tensor("S1", [128, 2048], f32).ap()
    tmp = nc.alloc_sbuf_tensor("tmp", [128, 2048], f32).ap()
    qw = nc.alloc_sbuf_tensor("qw", [128, 4096], f32).ap()
    ot = [nc.alloc_sbuf_tensor(f"ot{t}", [128, D], f32).ap() for t in range(2)]

    s_in = nc.alloc_semaphore("s_in")
    s_vec = nc.alloc_semaphore("s_vec")
    s_out = nc.alloc_semaphore("s_out")

    a_bc = abt[:, 0:64, None].broadcast_to([128, 64, 32])
    b_bc = abt[:, 64:128, None].broadcast_to([128, 64, 32])

    nc.gpsimd.dma_start(out=abt, in_=ab[:, :]).then_inc(s_in, 16)
    for t in range(2):
        rows = slice(t * 128, (t + 1) * 128)
        nc.gpsimd.dma_start(out=xt[t], in_=x[rows, :]).then_inc(s_in, 16)

    V = nc.vector
    for t in range(2):
        xn = xt[t].rearrange("p (k g h d) -> p k g h d", k=4, g=8, h=8, d=64)
        xs = xt[t].rearrange("p (k g h d) -> p k h g d", k=4, g=8, h=8, d=64)
        first = True
        for dh in range(2):
            ds_ = slice(dh * 32, (dh + 1) * 32)
            An = [xn[:, k, :, :, ds_] for k in range(4)]
            As = [xs[:, k, :, :, ds_] for k in range(4)]

            def nv(w_):
                return w_.rearrange("p (g h d) -> p g h d", g=8, h=8, d=32)

            def sv(w_):
                return w_.rearrange("p (g h d) -> p h g d", g=8, h=8, d=32)

            i0 = V.tensor_tensor(nv(C1), An[2], An[3], op=AL.add)
            if first:
                # gate tile compute on its input DMA (+ab on first tile)
                i0._wait_ge(s_in, 16 * (t + 2))
                first = False
            V.tensor_tensor(nv(S1), An[3], As[2], op=AL.mult)      # C2*A2^s
            V.tensor_tensor(nv(C0), An[1], nv(C1), op=AL.add)
            V.tensor_tensor(nv(tmp), nv(C1), As[1], op=AL.mult)    # C1*A1^s
            V.tensor_tensor(S1, S1, tmp, op=AL.add)
            V.tensor_tensor(nv(Tb), An[0], nv(C0), op=AL.add)
            V.tensor_tensor(nv(tmp), nv(C0), As[0], op=AL.mult)    # C0*A0^s
            V.tensor_tensor(S1, S1, tmp, op=AL.add)
            V.tensor_tensor(nv(tmp), nv(Tb), sv(Tb), op=AL.mult)   # T*T^s
            V.tensor_tensor(
                qw[:, 0:2048].rearrange("p (c d) -> p c d", c=64, d=32),
                a_bc, tmp.rearrange("p (c d) -> p c d", c=64, d=32), op=AL.mult)
            V.tensor_tensor(
                qw[:, 2048:4096].rearrange("p (c d) -> p c d", c=64, d=32),
                b_bc, S1.rearrange("p (c d) -> p c d", c=64, d=32), op=AL.mult)
            red = V.tensor_reduce(
                out=ot[t][:, ds_],
                in_=qw.rearrange("p (c d) -> p d c", c=128, d=32),
                axis=AX.X, op=AL.add)
            if dh == 1:
                red.then_inc(s_vec, 1)

    for t in range(2):
        rows = slice(t * 128, (t + 1) * 128)
        (nc.gpsimd.dma_start(out=out[rows, :], in_=ot[t])
         ._wait_ge(s_vec, t + 1).then_inc(s_out, 16))
    nc.gpsimd.wait_ge(s_out, 32)
    return nc


def _weights_ab(correlation: np.ndarray) -> np.ndarray:
    w = np.asarray(correlation, dtype=np.float32).reshape(G, G)
    gi = np.arange(G)[:, None]
    gj = np.arange(G)[None, :]
    alpha = np.where(gi < gj, w, 0.0).astype(np.float32)
    beta = (np.where(gi < gj, w.T - w, 0.0) + np.diag(np.diag(w))).astype(np.float32)
    row = np.concatenate([alpha.ravel(), beta.ravel()])
    return np.ascontiguousarray(np.broadcast_to(row, (128, 128)), dtype=np.float32)


def kernel(inputs: np.ndarray, correlation: np.ndarray, _trace: bool = False):
    _import_concourse()
    from concourse.bass_utils import run_bass_kernel_spmd

    if "nc" not in _CACHE:
        _CACHE["nc"] = _build()
    nc = _CACHE["nc"]

    x = np.ascontiguousarray(np.asarray(inputs, dtype=np.float32)).reshape(B, ROWS)
    ab = _weights_ab(correlation)
    in_maps = [{"x": x[c * BC:(c + 1) * BC], "ab": ab} for c in range(NCORES)]
    res = run_bass_kernel_spmd(nc, in_maps, core_ids=list(range(NCORES)),
                               trace=_trace)
    out = np.concatenate([r["out"] for r in res.results], axis=0)
    if _trace:
        return out, res
    return out
